# revision 15
# baseline (speedup 1.0000x reference)
"""Trainium2 Bass kernel for CropConv: 3x3 same-padding conv (64->64 ch) on
[16, 64, 128, 128] fp32 input, with a static crop mask zeroing output rows/cols
[44:84).

Strategy (data-parallel over batch, 8 cores x 2 images each):
  - Host marshals x into a zero-padded row-major layout with row stride 129
    (131 padded rows; the left zero column of each row doubles as the previous
    row's right pad), bf16.
  - Per core, image 0 lives in SBUF partitions 0-63 (partition = in-channel),
    image 1 in partitions 64-127.
  - Output rows are processed in 4-row chunks (32 per image).  Each chunk is
    9 PSUM-accumulated TensorE matmuls (one per conv tap): free dim is a
    [4 rows x 128 cols] strided access pattern (512 elements = exactly one
    PSUM bank), skipping the pad column.  K = M = 64, so four matmuls run
    concurrently in the four 64x64 PE quadrants: row-half = image, col-half =
    chunk pairing (u, u+16) (= y row halves 0..64 / 64..128).
  - Output path is bf16: PSUM -> SBUF stage copies convert to bf16 (one
    128-partition copy per image per unit), and the DRAM output layout IS the
    stage layout (partition p = rowhalf*64 + oc, free = (row, img, col)
    linear), so each store is a [128, 2048] fully contiguous DMA.  Stores
    fire every 2 units, overlapping compute.
  - Loads are issued on one queue in consumption-priority order (weights,
    then small lead segments for both row fronts) so compute starts early.
  - No on-device masking: the host zeroes the static crop window and converts
    bf16 -> fp32 while unpacking the stage layout.
"""

import numpy as np

# ---- problem constants (hardcoded; kernel.py must be self-contained) ----
B, C, H, W = 16, 64, 128, 128
OC, KS = 64, 3
N_CORES = 8
IMGS = B // N_CORES  # 2 images per core

WP = W + 1            # padded row stride: 129
HP = H + 3            # padded rows in the x buffer: 131
XLEN = HP * WP        # 16899 bf16 per partition

RPC = 4               # output rows per chunk
NCHK = H // RPC       # 32 chunks per image
NUNIT = NCHK // 2     # 16 pair units (u, u+16)
FREE = RPC * W        # matmul free size: 512 (= 1 PSUM bank of fp32)

# stage / output layout: [128, 64*2*128] bf16.
#   partition p = s*64 + oc, s = row half (0: y rows 0..64, 1: y rows 64..128)
#   free idx    = (r*2 + b)*128 + w   (r = row within half, b = image)
ROWH = H // 2         # 64 rows per half
RB = 2 * W            # free stride per stage row: 256
STLEN = ROWH * RB     # 16384

_CACHE = {}


def _build_module():
    import concourse.tile as tile
    from concourse import bacc, mybir

    f32 = mybir.dt.float32
    bf16 = mybir.dt.bfloat16

    nc = bacc.Bacc("TRN2", target_bir_lowering=False, debug=False,
                   num_devices=N_CORES)

    x_ap = nc.dram_tensor("xin", [IMGS, C, XLEN], bf16,
                          kind="ExternalInput").ap()
    w_ap = nc.dram_tensor("wt", [2 * C, KS * KS, OC], bf16,
                          kind="ExternalInput").ap()
    y_ap = nc.dram_tensor("yout", [128, STLEN], bf16,
                          kind="ExternalOutput").ap()

    x_bc = x_ap.rearrange("b c l -> (b c) l")  # [128, XLEN]

    with tile.TileContext(nc) as tc:
        with tc.tile_pool(name="big", bufs=1) as big, \
             tc.tile_pool(name="psum", bufs=8, space="PSUM") as pp:

            x_sb = big.tile([128, XLEN], bf16, tag="xbuf")
            stage = big.tile([128, STLEN], bf16, tag="stage")
            w_sb = big.tile([128, KS * KS * OC], bf16, tag="wbuf")

            st = stage.rearrange("p (r b w) -> p r b w", b=IMGS, w=W)
            # row views of x: x4 covers cols j = w+kw for kw in {0,1}; x4b is
            # shifted +2 so its row R covers j = w+2 (the kw=2 tap), where
            # col 129 of a row = the next row's zero left-pad (right-pad trick)
            x4 = x_sb.rearrange("p (r j) -> p r j", j=WP)          # [.,131,129]
            x4b = x_sb[:, 2:2 + 130 * WP].rearrange(
                "p (r j) -> p r j", j=WP)                          # [.,130,129]

            # prioritized loads: weights first (tiny, host-replicated to both
            # halves), then small lead segments of both consumption fronts
            # (upper lead on the otherwise-idle gpsimd queue so its trigger
            # doesn't serialize behind the sync queue), then the rest
            w_flat = w_ap.rearrange("i t o -> i (t o)")  # [128, 1152]
            nc.sync.dma_start(out=w_sb[:, :], in_=w_flat)
            nc.gpsimd.dma_start(out=x_sb[:, 64 * WP:70 * WP],
                                in_=x_bc[:, 64 * WP:70 * WP])

            segs = [(0, 6), (6, 15), (70, 79), (15, 24), (79, 88),
                    (24, 44), (88, 108), (44, 64), (108, 131)]
            for (a, b_) in segs:
                nc.sync.dma_start(out=x_sb[:, a * WP:b_ * WP],
                                  in_=x_bc[:, a * WP:b_ * WP])

            def lhsT(half, t):
                return w_sb[half * 64:(half + 1) * 64, t * OC:(t + 1) * OC]

            def rhs(half, c, kh, kw):
                h0, h1 = half * 64, (half + 1) * 64
                R = RPC * c + kh
                if kw == 2:
                    return x4b[h0:h1, R:R + RPC, 0:W]
                return x4[h0:h1, R:R + RPC, kw:kw + W]

            TAPS = [(kh, kw) for kh in range(KS) for kw in range(KS)]

            for u in range(NUNIT):
                c2 = u + NCHK // 2
                pa = pp.tile([128, FREE], f32, tag="ps")
                pb = pp.tile([128, FREE], f32, tag="ps")
                for t, (kh, kw) in enumerate(TAPS):
                    sta, sp = (t == 0), (t == len(TAPS) - 1)
                    nc.tensor.matmul(pa[0:64, :], lhsT(0, t),
                                     rhs(0, u, kh, kw), start=sta,
                                     stop=sp, skip_group_check=True)
                    nc.tensor.matmul(pa[64:128, :], lhsT(0, t),
                                     rhs(0, c2, kh, kw), start=sta,
                                     stop=sp, skip_group_check=True)
                    nc.tensor.matmul(pb[0:64, :], lhsT(1, t),
                                     rhs(1, u, kh, kw), start=sta,
                                     stop=sp, skip_group_check=True)
                    nc.tensor.matmul(pb[64:128, :], lhsT(1, t),
                                     rhs(1, c2, kh, kw), start=sta,
                                     stop=sp, skip_group_check=True)
                r0 = RPC * u
                for img, pt in ((0, pa), (1, pb)):
                    pe = pt[:, :].rearrange("p (h w) -> p h w", w=W)
                    nc.any.tensor_copy(st[:, r0:r0 + RPC, img, :],
                                       pe[:, 0:RPC, :])

                # stores: 8-row granules [8g, 8g+8) of both halves as one
                # fully-contiguous [128, 2048] DMA; finer 4-row stores at the
                # end to shrink the exposed tail
                bounds = None
                if u % 2 == 1 and u <= 11:
                    bounds = (8 * (u // 2), 8 * (u // 2) + 8)
                elif u == 13:
                    bounds = (48, 56)
                elif u >= 14:
                    bounds = (RPC * u, RPC * (u + 1))
                if bounds:
                    o0, o1 = bounds[0] * RB, bounds[1] * RB
                    nc.gpsimd.dma_start(out=y_ap[:, o0:o1],
                                        in_=stage[:, o0:o1])

    nc.compile()
    return nc


def _get_module():
    if "nc" not in _CACHE:
        _CACHE["nc"] = _build_module()
    return _CACHE["nc"]


def _make_in_maps(x, weight):
    x = np.asarray(x, dtype=np.float32)
    weight = np.asarray(weight, dtype=np.float32)
    # host marshaling: pad x into the row-major stride-129 layout
    xp = np.zeros((B, C, HP, WP), dtype=np.float32)
    xp[:, :, 1:H + 1, 1:W + 1] = x
    xp = xp.reshape(B, C, XLEN)
    import ml_dtypes
    xp = xp.astype(ml_dtypes.bfloat16)
    # weight [oc, ic, kh, kw] -> [ic, (kh kw), oc], replicated to both halves
    w1 = weight.transpose(1, 2, 3, 0).reshape(C, KS * KS, OC)
    wt = np.ascontiguousarray(
        np.concatenate([w1, w1], axis=0)).astype(ml_dtypes.bfloat16)
    return [
        {"xin": np.ascontiguousarray(xp[k * IMGS:(k + 1) * IMGS]), "wt": wt}
        for k in range(N_CORES)
    ]


def kernel(x, weight):
    from concourse.bass_utils import run_bass_kernel_spmd

    nc = _get_module()
    in_maps = _make_in_maps(x, weight)
    res = run_bass_kernel_spmd(nc, in_maps, list(range(N_CORES)))
    out = np.empty((B, OC, H, W), dtype=np.float32)
    for k in range(N_CORES):
        a = np.asarray(res.results[k]["yout"]).reshape(128, ROWH, IMGS, W)
        blk = out[k * IMGS:(k + 1) * IMGS]
        # [oc, r, b, w] -> [b, oc, r, w]
        blk[:, :, 0:ROWH] = a[0:64].transpose(2, 0, 1, 3).astype(np.float32)
        blk[:, :, ROWH:H] = a[64:128].transpose(2, 0, 1, 3).astype(np.float32)
    # static crop mask: host zeroes rows/cols [44:84)
    out[:, :, 44:84, 44:84] = 0.0
    return out


# revision 16
# speedup vs baseline: 1.0270x; 1.0270x over previous
"""Trainium2 Bass kernel for CropConv: 3x3 same-padding conv (64->64 ch) on
[16, 64, 128, 128] fp32 input, with a static crop mask zeroing output rows/cols
[44:84).

Strategy (data-parallel over batch, 8 cores x 2 images each):
  - Host marshals x into a zero-padded row-major layout with row stride 129
    (131 padded rows; the left zero column of each row doubles as the previous
    row's right pad), bf16.
  - Per core, image 0 lives in SBUF partitions 0-63 (partition = in-channel),
    image 1 in partitions 64-127.
  - Output rows are processed in 4-row chunks (32 per image).  Each chunk is
    9 PSUM-accumulated TensorE matmuls (one per conv tap): free dim is a
    [4 rows x 128 cols] strided access pattern (512 elements = exactly one
    PSUM bank), skipping the pad column.  K = M = 64, so four matmuls run
    concurrently in the four 64x64 PE quadrants: row-half = image, col-half =
    chunk pairing (u, u+16) (= y row halves 0..64 / 64..128).
  - Output path is bf16: PSUM -> SBUF stage copies convert to bf16 (one
    128-partition copy per image per unit), and the DRAM output layout IS the
    stage layout (partition p = rowhalf*64 + oc, free = (row, img, col)
    linear), so each store is a [128, 2048] fully contiguous DMA.  Stores
    fire every 2 units, overlapping compute.
  - Loads are issued on one queue in consumption-priority order (weights,
    then small lead segments for both row fronts) so compute starts early.
  - No on-device masking: the host zeroes the static crop window and converts
    bf16 -> fp32 while unpacking the stage layout.
"""

import numpy as np

# ---- problem constants (hardcoded; kernel.py must be self-contained) ----
B, C, H, W = 16, 64, 128, 128
OC, KS = 64, 3
N_CORES = 8
IMGS = B // N_CORES  # 2 images per core

WP = W + 1            # padded row stride: 129
HP = H + 3            # padded rows in the x buffer: 131
XLEN = HP * WP        # 16899 bf16 per partition

RPC = 4               # output rows per chunk
NCHK = H // RPC       # 32 chunks per image
NUNIT = NCHK // 2     # 16 pair units (u, u+16)
FREE = RPC * W        # matmul free size: 512 (= 1 PSUM bank of fp32)

# stage / output layout: [128, 64*2*128] bf16.
#   partition p = s*64 + oc, s = row half (0: y rows 0..64, 1: y rows 64..128)
#   free idx    = (r*2 + b)*128 + w   (r = row within half, b = image)
ROWH = H // 2         # 64 rows per half
RB = 2 * W            # free stride per stage row: 256
STLEN = ROWH * RB     # 16384

_CACHE = {}


def _build_module():
    import concourse.tile as tile
    from concourse import bacc, mybir

    f32 = mybir.dt.float32
    bf16 = mybir.dt.bfloat16

    nc = bacc.Bacc("TRN2", target_bir_lowering=False, debug=False,
                   num_devices=N_CORES)

    x_ap = nc.dram_tensor("xin", [IMGS, C, XLEN], bf16,
                          kind="ExternalInput").ap()
    w_ap = nc.dram_tensor("wt", [2 * C, KS * KS, OC], bf16,
                          kind="ExternalInput").ap()
    y_ap = nc.dram_tensor("yout", [128, STLEN], bf16,
                          kind="ExternalOutput").ap()

    x_bc = x_ap.rearrange("b c l -> (b c) l")  # [128, XLEN]

    with tile.TileContext(nc) as tc:
        with tc.tile_pool(name="big", bufs=1) as big, \
             tc.tile_pool(name="psum", bufs=8, space="PSUM") as pp:

            x_sb = big.tile([128, XLEN], bf16, tag="xbuf")
            stage = big.tile([128, STLEN], bf16, tag="stage")
            w_sb = big.tile([128, KS * KS * OC], bf16, tag="wbuf")

            st = stage.rearrange("p (r b w) -> p r b w", b=IMGS, w=W)
            # row views of x: x4 covers cols j = w+kw for kw in {0,1}; x4b is
            # shifted +2 so its row R covers j = w+2 (the kw=2 tap), where
            # col 129 of a row = the next row's zero left-pad (right-pad trick)
            x4 = x_sb.rearrange("p (r j) -> p r j", j=WP)          # [.,131,129]
            x4b = x_sb[:, 2:2 + 130 * WP].rearrange(
                "p (r j) -> p r j", j=WP)                          # [.,130,129]

            # prioritized loads: weights first (tiny, host-replicated to both
            # halves), then small lead segments of both consumption fronts
            # (upper lead on the otherwise-idle gpsimd queue so its trigger
            # doesn't serialize behind the sync queue), then the rest
            w_flat = w_ap.rearrange("i t o -> i (t o)")  # [128, 1152]
            nc.sync.dma_start(out=w_sb[:, :], in_=w_flat)
            nc.gpsimd.dma_start(out=x_sb[:, 64 * WP:71 * WP],
                                in_=x_bc[:, 64 * WP:71 * WP])

            segs = [(0, 7), (7, 24), (71, 88), (24, 44),
                    (88, 108), (44, 64), (108, 131)]
            for (a, b_) in segs:
                nc.sync.dma_start(out=x_sb[:, a * WP:b_ * WP],
                                  in_=x_bc[:, a * WP:b_ * WP])

            def lhsT(half, t):
                return w_sb[half * 64:(half + 1) * 64, t * OC:(t + 1) * OC]

            def rhs(half, c, kh, kw):
                h0, h1 = half * 64, (half + 1) * 64
                R = RPC * c + kh
                if kw == 2:
                    return x4b[h0:h1, R:R + RPC, 0:W]
                return x4[h0:h1, R:R + RPC, kw:kw + W]

            TAPS = [(kh, kw) for kh in range(KS) for kw in range(KS)]

            for u in range(NUNIT):
                c2 = u + NCHK // 2
                pa = pp.tile([128, FREE], f32, tag="ps")
                pb = pp.tile([128, FREE], f32, tag="ps")
                for t, (kh, kw) in enumerate(TAPS):
                    sta, sp = (t == 0), (t == len(TAPS) - 1)
                    nc.tensor.matmul(pa[0:64, :], lhsT(0, t),
                                     rhs(0, u, kh, kw), start=sta,
                                     stop=sp, skip_group_check=True)
                    nc.tensor.matmul(pa[64:128, :], lhsT(0, t),
                                     rhs(0, c2, kh, kw), start=sta,
                                     stop=sp, skip_group_check=True)
                    nc.tensor.matmul(pb[0:64, :], lhsT(1, t),
                                     rhs(1, u, kh, kw), start=sta,
                                     stop=sp, skip_group_check=True)
                    nc.tensor.matmul(pb[64:128, :], lhsT(1, t),
                                     rhs(1, c2, kh, kw), start=sta,
                                     stop=sp, skip_group_check=True)
                r0 = RPC * u
                for img, pt in ((0, pa), (1, pb)):
                    pe = pt[:, :].rearrange("p (h w) -> p h w", w=W)
                    nc.any.tensor_copy(st[:, r0:r0 + RPC, img, :],
                                       pe[:, 0:RPC, :])

                # stores: 8-row granules [8g, 8g+8) of both halves as one
                # fully-contiguous [128, 2048] DMA; finer 4-row stores at the
                # end to shrink the exposed tail
                bounds = None
                if u % 2 == 1 and u <= 11:
                    bounds = (8 * (u // 2), 8 * (u // 2) + 8)
                elif u == 13:
                    bounds = (48, 56)
                elif u >= 14:
                    bounds = (RPC * u, RPC * (u + 1))
                if bounds:
                    o0, o1 = bounds[0] * RB, bounds[1] * RB
                    nc.scalar.dma_start(out=y_ap[:, o0:o1],
                                        in_=stage[:, o0:o1])

    nc.compile()
    return nc


def _get_module():
    if "nc" not in _CACHE:
        _CACHE["nc"] = _build_module()
    return _CACHE["nc"]


def _make_in_maps(x, weight):
    x = np.asarray(x, dtype=np.float32)
    weight = np.asarray(weight, dtype=np.float32)
    # host marshaling: pad x into the row-major stride-129 layout
    xp = np.zeros((B, C, HP, WP), dtype=np.float32)
    xp[:, :, 1:H + 1, 1:W + 1] = x
    xp = xp.reshape(B, C, XLEN)
    import ml_dtypes
    xp = xp.astype(ml_dtypes.bfloat16)
    # weight [oc, ic, kh, kw] -> [ic, (kh kw), oc], replicated to both halves
    w1 = weight.transpose(1, 2, 3, 0).reshape(C, KS * KS, OC)
    wt = np.ascontiguousarray(
        np.concatenate([w1, w1], axis=0)).astype(ml_dtypes.bfloat16)
    return [
        {"xin": np.ascontiguousarray(xp[k * IMGS:(k + 1) * IMGS]), "wt": wt}
        for k in range(N_CORES)
    ]


def kernel(x, weight):
    from concourse.bass_utils import run_bass_kernel_spmd

    nc = _get_module()
    in_maps = _make_in_maps(x, weight)
    res = run_bass_kernel_spmd(nc, in_maps, list(range(N_CORES)))
    out = np.empty((B, OC, H, W), dtype=np.float32)
    for k in range(N_CORES):
        a = np.asarray(res.results[k]["yout"]).reshape(128, ROWH, IMGS, W)
        blk = out[k * IMGS:(k + 1) * IMGS]
        # [oc, r, b, w] -> [b, oc, r, w]
        blk[:, :, 0:ROWH] = a[0:64].transpose(2, 0, 1, 3).astype(np.float32)
        blk[:, :, ROWH:H] = a[64:128].transpose(2, 0, 1, 3).astype(np.float32)
    # static crop mask: host zeroes rows/cols [44:84)
    out[:, :, 44:84, 44:84] = 0.0
    return out
